# revision 3
# baseline (speedup 1.0000x reference)
"""GCN layer on 8 Trainium2 NeuronCores.

out = relu(A @ (X @ W) + b) computed as relu((A @ X) @ W + b) (linearity),
where A is the sparse COO matrix (edge_row <- edge_col, edge_val).

Sharding: dest rows (output) split contiguously across 8 cores (12500 each).
Edges bucketed by (core, dest-block of 128 rows, source-chunk of 25000 rows);
each bucket padded to a uniform size so one SPMD program serves all cores.

Per core, per dest-block:
  - dma_gather (SWDGE) pulls the block's edge source rows X[col] into SBUF
    (4 gathers, one per source chunk; int16 chunk-local indices)
  - for each 128-edge tile: DVE builds S^T[e,d] = val[e] * (d == row_local[e])
    via one tensor_scalar(is_equal, mult) against a constant IOTA; PE matmul
    accumulates agg[d,f] += S^T.T @ G into PSUM
  - epilogue: PE-transpose agg, 2 matmuls with W + K=1 bias matmul, ACT relu,
    DMA out
"""
import sys
import numpy as np

sys.path.insert(0, '/opt/trn_rl_repo')

import concourse.bass as bass          # noqa: E402
import concourse.bacc as bacc          # noqa: E402
import concourse.mybir as mybir        # noqa: E402
import concourse.tile as tile          # noqa: E402
from concourse.bass_utils import run_bass_kernel_spmd  # noqa: E402

N_NODES = 100000
N_EDGES = 3200000
D = 256
N_CORES = 8
P = 128
ROWS_PER_CORE = N_NODES // N_CORES          # 12500
NB = (ROWS_PER_CORE + P - 1) // P           # 98 dest blocks (last has 84 rows)
LAST_ROWS = ROWS_PER_CORE - (NB - 1) * P    # 84
NCHUNK = 5
CHUNK = N_NODES // NCHUNK                   # 20000 (< int16 max)
GMAX_TILES = 8                              # dma_gather hard limit: 1024 idxs
f32 = mybir.dt.float32
i16 = mybir.dt.int16

_cache = {}

# test-harness hooks (harness only calls kernel(); these stay inert there)
TRACE_TMPDIR = None          # set to a dir path to capture a perfetto trace
LAST_RESULT = None           # BassKernelResults of the most recent run


def _build_program(t_bkt: int):
    """One SPMD program; t_bkt = tiles (of 128 edges) per (block, chunk) bucket."""
    s_blk = NCHUNK * t_bkt          # tiles per dest block
    e_bkt = t_bkt * P               # edges per bucket

    nc = bacc.Bacc("TRN2", target_bir_lowering=False)
    X = nc.dram_tensor("X", [N_NODES, D], f32, kind="ExternalInput")
    Wt = nc.dram_tensor("Wt", [P, 2, D], f32, kind="ExternalInput")   # W[k*128+p, d] at [p, k, d]
    Bb = nc.dram_tensor("Bb", [1, D], f32, kind="ExternalInput")
    ONES = nc.dram_tensor("ONES", [1, P], f32, kind="ExternalInput")
    IOTA = nc.dram_tensor("IOTA", [P, P], f32, kind="ExternalInput")
    IDENT = nc.dram_tensor("IDENT", [P, P], f32, kind="ExternalInput")
    META = nc.dram_tensor("META", [P, NB * 2 * s_blk], f32, kind="ExternalInput")
    COLIDX = nc.dram_tensor("COLIDX", [P, NB * s_blk * 8], i16, kind="ExternalInput")
    OUT = nc.dram_tensor("OUT", [ROWS_PER_CORE, D], f32, kind="ExternalOutput")

    with tile.TileContext(nc) as tc:
        with (
            tc.tile_pool(name="const", bufs=1) as const_pool,
            tc.tile_pool(name="meta", bufs=3) as meta_pool,
            tc.tile_pool(name="idx", bufs=3) as idx_pool,
            tc.tile_pool(name="g", bufs=3) as g_pool,
            tc.tile_pool(name="st", bufs=6) as st_pool,
            tc.tile_pool(name="agg", bufs=2, space="PSUM") as agg_pool,
            tc.tile_pool(name="tp", bufs=2, space="PSUM") as tp_pool,
            tc.tile_pool(name="op", bufs=2, space="PSUM") as op_pool,
            tc.tile_pool(name="sb", bufs=3) as sb_pool,
        ):
            w_t = const_pool.tile([P, 2, D], f32, tag="w")
            nc.sync.dma_start(out=w_t[:], in_=Wt[:, :, :])
            bias_t = const_pool.tile([1, D], f32, tag="bias")
            nc.sync.dma_start(out=bias_t[:], in_=Bb[:, :])
            ones_t = const_pool.tile([1, P], f32, tag="ones")
            nc.sync.dma_start(out=ones_t[:], in_=ONES[:, :])
            iota_t = const_pool.tile([P, P], f32, tag="iota")
            nc.sync.dma_start(out=iota_t[:], in_=IOTA[:, :])
            ident_t = const_pool.tile([P, P], f32, tag="ident")
            nc.sync.dma_start(out=ident_t[:], in_=IDENT[:, :])

            for d in range(NB):
                meta_t = meta_pool.tile([P, 2 * s_blk], f32, tag="meta")
                nc.sync.dma_start(
                    out=meta_t[:], in_=META[:, d * 2 * s_blk:(d + 1) * 2 * s_blk])
                idx_t = idx_pool.tile([P, s_blk * 8], i16, tag="idx")
                nc.sync.dma_start(
                    out=idx_t[:], in_=COLIDX[:, d * s_blk * 8:(d + 1) * s_blk * 8])

                g_t = g_pool.tile([P, s_blk, D], f32, tag="g")
                for c in range(NCHUNK):
                    for t0 in range(0, t_bkt, GMAX_TILES):
                        tn = min(GMAX_TILES, t_bkt - t0)
                        nc.gpsimd.dma_gather(
                            out_ap=g_t[:, c * t_bkt + t0:c * t_bkt + t0 + tn, :],
                            in_ap=X[c * CHUNK:(c + 1) * CHUNK, :],
                            idxs_ap=idx_t[:, (c * t_bkt + t0) * 8:(c * t_bkt + t0 + tn) * 8],
                            num_idxs=tn * P,
                            num_idxs_reg=tn * P,
                            elem_size=D,
                        )

                agg_psum = agg_pool.tile([P, D], f32, tag="agg", space="PSUM")
                for s in range(s_blk):
                    s_t = st_pool.tile([P, P], f32, tag="st")
                    nc.vector.tensor_scalar(
                        out=s_t[:],
                        in0=iota_t[:],
                        scalar1=meta_t[:, s:s + 1],
                        scalar2=meta_t[:, s_blk + s:s_blk + s + 1],
                        op0=mybir.AluOpType.is_equal,
                        op1=mybir.AluOpType.mult,
                    )
                    nc.tensor.matmul(
                        out=agg_psum[:],
                        lhsT=s_t[:],
                        rhs=g_t[:, s, :],
                        start=(s == 0),
                        stop=(s == s_blk - 1),
                    )

                # epilogue: agg -> SBUF, transpose, @W + bias, relu, out
                agg_sb = sb_pool.tile([P, D], f32, tag="aggsb")
                nc.vector.tensor_copy(out=agg_sb[:], in_=agg_psum[:])
                aggT_sb = sb_pool.tile([P, 2, P], f32, tag="aggT")
                for k in range(2):
                    tp = tp_pool.tile([P, P], f32, tag="tp", space="PSUM")
                    nc.tensor.transpose(
                        out=tp[:], in_=agg_sb[:, k * P:(k + 1) * P],
                        identity=ident_t[:])
                    nc.vector.tensor_copy(out=aggT_sb[:, k, :], in_=tp[:])

                out_psum = op_pool.tile([P, D], f32, tag="outp", space="PSUM")
                for k in range(2):
                    nc.tensor.matmul(
                        out=out_psum[:], lhsT=aggT_sb[:, k, :], rhs=w_t[:, k, :],
                        start=(k == 0), stop=False)
                nc.tensor.matmul(
                    out=out_psum[:], lhsT=ones_t[:], rhs=bias_t[:],
                    start=False, stop=True)

                rows = P if d < NB - 1 else LAST_ROWS
                osb = sb_pool.tile([P, D], f32, tag="osb")
                nc.scalar.activation(
                    out=osb[:], in_=out_psum[:],
                    func=mybir.ActivationFunctionType.Relu)
                nc.sync.dma_start(
                    out=OUT[d * P:d * P + rows, :], in_=osb[:rows, :])
    nc.compile()
    return nc


def _preprocess(edge_row, edge_col, edge_val):
    """Bucket edges by (core, dest block, source chunk); pad uniformly.

    Returns t_bkt and per-core (META [P, NB*2*s_blk] f32,
    COLIDX [P, NB*s_blk*8] int16).
    """
    r = np.asarray(edge_row).astype(np.int64)
    c = np.asarray(edge_col).astype(np.int64)
    v = np.asarray(edge_val).astype(np.float32)

    core = r // ROWS_PER_CORE
    r_loc = r - core * ROWS_PER_CORE
    blk = r_loc // P
    rib = (r_loc - blk * P).astype(np.float32)
    chunk = c // CHUNK
    c_loc = (c - chunk * CHUNK).astype(np.int16)

    key = ((core * NB + blk) * NCHUNK + chunk).astype(np.int64)
    nbuckets = N_CORES * NB * NCHUNK
    counts = np.bincount(key, minlength=nbuckets)
    e_bkt = int(-(-counts.max() // P) * P)
    t_bkt = e_bkt // P
    s_blk = NCHUNK * t_bkt

    order = np.argsort(key, kind='stable')
    key_sorted = key[order]
    starts = np.zeros(nbuckets, np.int64)
    np.cumsum(counts[:-1], out=starts[1:])
    rank = np.arange(N_EDGES, dtype=np.int64) - starts[key_sorted]
    pos = key_sorted * e_bkt + rank   # position in global padded array

    tot = nbuckets * e_bkt
    col_pad = np.zeros(tot, np.int16)
    val_pad = np.zeros(tot, np.float32)
    rib_pad = np.zeros(tot, np.float32)
    col_pad[pos] = c_loc[order]
    val_pad[pos] = v[order]
    rib_pad[pos] = rib[order]

    # reshape to per-core device layouts
    col_pad = col_pad.reshape(N_CORES, NB, s_blk * P)
    val_pad = val_pad.reshape(N_CORES, NB, s_blk, P)
    rib_pad = rib_pad.reshape(N_CORES, NB, s_blk, P)

    metas, colidxs = [], []
    for cc in range(N_CORES):
        # META: per block [rows(s_blk) | vals(s_blk)] ; [p, ...] = edge s*128+p
        m = np.empty((P, NB, 2, s_blk), np.float32)
        m[:, :, 0, :] = rib_pad[cc].transpose(2, 0, 1)
        m[:, :, 1, :] = val_pad[cc].transpose(2, 0, 1)
        metas.append(np.ascontiguousarray(m.reshape(P, NB * 2 * s_blk)))
        # COLIDX: idx i of a bucket -> partition i%16 (replicated x8), slot i//16
        ci = col_pad[cc].reshape(NB * s_blk * 8, 16).T      # [16, NB*s_blk*8]
        ci = np.broadcast_to(ci[None, :, :], (8, 16, NB * s_blk * 8))
        colidxs.append(np.ascontiguousarray(ci.reshape(P, NB * s_blk * 8)))
    return t_bkt, metas, colidxs


def kernel(X, edge_row, edge_col, edge_val, W, b):
    X = np.ascontiguousarray(np.asarray(X, dtype=np.float32))
    W = np.asarray(W, dtype=np.float32)
    b = np.asarray(b, dtype=np.float32)

    t_bkt, metas, colidxs = _preprocess(edge_row, edge_col, edge_val)
    if t_bkt not in _cache:
        _cache[t_bkt] = _build_program(t_bkt)
    nc = _cache[t_bkt]

    w_rs = np.ascontiguousarray(W.reshape(2, P, D).transpose(1, 0, 2))
    bias = b.reshape(1, D).copy()
    ones = np.ones((1, P), np.float32)
    iota = np.broadcast_to(np.arange(P, dtype=np.float32), (P, P)).copy()
    ident = np.eye(P, dtype=np.float32)

    in_maps = []
    for cc in range(N_CORES):
        in_maps.append({
            "X": X, "Wt": w_rs, "Bb": bias, "ONES": ones,
            "IOTA": iota, "IDENT": ident,
            "META": metas[cc], "COLIDX": colidxs[cc],
        })
    kw = {}
    if TRACE_TMPDIR:
        kw = dict(trace=True, tmpdir=TRACE_TMPDIR)
    res = run_bass_kernel_spmd(nc, in_maps, core_ids=list(range(N_CORES)), **kw)
    global LAST_RESULT
    LAST_RESULT = res
    return np.concatenate([res.results[cc]["OUT"] for cc in range(N_CORES)], axis=0)



# revision 4
# speedup vs baseline: 1.0105x; 1.0105x over previous
"""GCN layer on 8 Trainium2 NeuronCores — v4s (col-sorted buckets).

out = relu(A @ (X @ W) + b) computed as relu((A @ X) @ W + b) (linearity),
where A is the sparse COO matrix (edge_row <- edge_col, edge_val).

Sharding: dest rows (output) split contiguously across 8 cores (12500 each).
Edges bucketed by (core, dest-block of 128 rows, source-chunk of 20000 rows);
each bucket padded to a uniform size so one SPMD program serves all cores.

v3 vs baseline:
  - X cast to fp16; gathers move 512B/edge instead of 1KB
  - dma_gather round-robined over 4 SWDGE queues (descriptor generation
    runs on a distinct GpSimd Q7 core pair per queue) -> ~3.4x DGE
  - scatter matrices S^T (one-hot(row) * val per 128-edge tile) are
    precomputed host-side and DMA'd per dest block, replacing the slow
    per-tile DVE tensor_scalar build (~900ns each, was 90% DVE busy)
  - bucket padding indices set to -1: SWDGE trims trailing negatives,
    skipping both descriptor-gen and transfer for pad slots
  - all matmuls fp16 (1 cyc/row vs 4 for fp32), accumulate fp32 in PSUM
  - per-gather valid-index counts fed via Pool registers (CNT tensor);
    with TRIM=1, pad indices are -1 and SWDGE skips them entirely
  - OUT stored fp16, upcast to fp32 on host
"""
import os
import sys
import numpy as np

sys.path.insert(0, '/opt/trn_rl_repo')

import concourse.bass as bass          # noqa: E402
import concourse.bacc as bacc          # noqa: E402
import concourse.mybir as mybir        # noqa: E402
import concourse.tile as tile          # noqa: E402
from concourse.bass_utils import run_bass_kernel_spmd  # noqa: E402

N_NODES = 100000
N_EDGES = 3200000
D = 256
N_CORES = 8
P = 128
ROWS_PER_CORE = N_NODES // N_CORES          # 12500
NB = (ROWS_PER_CORE + P - 1) // P           # 98 dest blocks (last has 84 rows)
LAST_ROWS = ROWS_PER_CORE - (NB - 1) * P    # 84
NCHUNK = 5
CHUNK = N_NODES // NCHUNK                   # 20000 (< int16 max)
GMAX_TILES = 8                              # dma_gather limit: 1024 idxs
NQ = 4                                      # SWDGE queues
f32 = mybir.dt.float32
f16 = mybir.dt.float16
i16 = mybir.dt.int16

_cache = {}

# test-harness hooks (harness only calls kernel(); these stay inert there)
TRACE_TMPDIR = None
LAST_RESULT = None


def _build_program(t_bkt: int):
    """One SPMD program; t_bkt = tiles (of 128 edges) per (block, chunk) bucket."""
    # CNT is indexed per (block, chunk) bucket, which assumes each bucket is
    # covered by a single dma_gather call.
    assert t_bkt <= GMAX_TILES, f"per-call CNT indexing requires t_bkt<={GMAX_TILES}"
    s_blk = NCHUNK * t_bkt          # tiles per dest block

    nc = bacc.Bacc("TRN2", target_bir_lowering=False, num_swdge_queues=NQ)
    X = nc.dram_tensor("X", [N_NODES, D], f16, kind="ExternalInput")
    Wt = nc.dram_tensor("Wt", [P, 2, D], f16, kind="ExternalInput")
    Bb = nc.dram_tensor("Bb", [1, D], f16, kind="ExternalInput")
    ONES = nc.dram_tensor("ONES", [1, P], f16, kind="ExternalInput")
    IDENT = nc.dram_tensor("IDENT", [P, P], f16, kind="ExternalInput")
    ST = nc.dram_tensor("ST", [P, NB * s_blk * P], f16, kind="ExternalInput")
    COLIDX = nc.dram_tensor("COLIDX", [P, NB * s_blk * 8], i16, kind="ExternalInput")
    CNT = nc.dram_tensor("CNT", [1, NB * NCHUNK], mybir.dt.int32, kind="ExternalInput")
    OUT = nc.dram_tensor("OUT", [ROWS_PER_CORE, D], f16, kind="ExternalOutput")

    qctr = [0]

    with tile.TileContext(nc) as tc:
        with (
            tc.tile_pool(name="const", bufs=1) as const_pool,
            tc.tile_pool(name="st", bufs=3) as st_pool,
            tc.tile_pool(name="idx", bufs=3) as idx_pool,
            tc.tile_pool(name="g", bufs=3) as g_pool,
            tc.tile_pool(name="agg", bufs=2, space="PSUM") as agg_pool,
            tc.tile_pool(name="tp", bufs=2, space="PSUM") as tp_pool,
            tc.tile_pool(name="op", bufs=2, space="PSUM") as op_pool,
            tc.tile_pool(name="sb", bufs=3) as sb_pool,
        ):
            w_t = const_pool.tile([P, 2, D], f16, tag="w")
            nc.sync.dma_start(out=w_t[:], in_=Wt[:, :, :])
            bias_t = const_pool.tile([1, D], f16, tag="bias")
            nc.sync.dma_start(out=bias_t[:], in_=Bb[:, :])
            ones_t = const_pool.tile([1, P], f16, tag="ones")
            nc.sync.dma_start(out=ones_t[:], in_=ONES[:, :])
            ident_t = const_pool.tile([P, P], f16, tag="ident")
            nc.sync.dma_start(out=ident_t[:], in_=IDENT[:, :])
            cnt_t = const_pool.tile([1, NB * NCHUNK], mybir.dt.int32, tag="cnt")
            nc.sync.dma_start(out=cnt_t[:], in_=CNT[:, :])
            cregs = [nc.alloc_register(mybir.EngineType.Pool, f"cnt{q}")
                     for q in range(NQ)]

            # zero-fill the gather buffers once: trailing-trimmed pad slots
            # are never written, and first-touch SBUF bytes could decode as
            # NaN in fp16 (NaN * 0 = NaN in the agg matmul).
            for _ in range(3):
                g_warm = g_pool.tile([P, s_blk, D], f16, tag="g")
                nc.vector.memset(g_warm[:], 0.0)

            for d in range(NB):
                st_t = st_pool.tile([P, s_blk, P], f16, tag="st")
                nc.sync.dma_start(
                    out=st_t[:],
                    in_=ST[:, d * s_blk * P:(d + 1) * s_blk * P])
                idx_t = idx_pool.tile([P, s_blk * 8], i16, tag="idx")
                nc.sync.dma_start(
                    out=idx_t[:], in_=COLIDX[:, d * s_blk * 8:(d + 1) * s_blk * 8])

                g_t = g_pool.tile([P, s_blk, D], f16, tag="g")
                for c in range(NCHUNK):
                    for t0 in range(0, t_bkt, GMAX_TILES):
                        tn = min(GMAX_TILES, t_bkt - t0)
                        q = qctr[0] % NQ
                        nc.gpsimd.reg_load(
                            cregs[q],
                            cnt_t[0:1, d * NCHUNK + c:d * NCHUNK + c + 1])
                        nc.gpsimd.dma_gather(
                            out_ap=g_t[:, c * t_bkt + t0:c * t_bkt + t0 + tn, :],
                            in_ap=X[c * CHUNK:(c + 1) * CHUNK, :],
                            idxs_ap=idx_t[:, (c * t_bkt + t0) * 8:(c * t_bkt + t0 + tn) * 8],
                            num_idxs=tn * P,
                            num_idxs_reg=cregs[q],
                            elem_size=D,
                            queue_num=q,
                        )
                        qctr[0] += 1

                agg_psum = agg_pool.tile([P, D], f32, tag="agg", space="PSUM")
                for s in range(s_blk):
                    nc.tensor.matmul(
                        out=agg_psum[:],
                        lhsT=st_t[:, s, :],
                        rhs=g_t[:, s, :],
                        start=(s == 0),
                        stop=(s == s_blk - 1),
                    )

                # epilogue: agg -> SBUF (f16), transpose, @W + bias, relu, out
                agg_sb = sb_pool.tile([P, D], f16, tag="aggsb")
                nc.vector.tensor_copy(out=agg_sb[:], in_=agg_psum[:])
                aggT_sb = sb_pool.tile([P, 2, P], f16, tag="aggT")
                for k in range(2):
                    tp = tp_pool.tile([P, P], f16, tag="tp", space="PSUM")
                    nc.tensor.transpose(
                        out=tp[:], in_=agg_sb[:, k * P:(k + 1) * P],
                        identity=ident_t[:])
                    nc.vector.tensor_copy(out=aggT_sb[:, k, :], in_=tp[:])

                out_psum = op_pool.tile([P, D], f32, tag="outp", space="PSUM")
                for k in range(2):
                    nc.tensor.matmul(
                        out=out_psum[:], lhsT=aggT_sb[:, k, :], rhs=w_t[:, k, :],
                        start=(k == 0), stop=False)
                nc.tensor.matmul(
                    out=out_psum[:], lhsT=ones_t[:], rhs=bias_t[:],
                    start=False, stop=True)

                rows = P if d < NB - 1 else LAST_ROWS
                osb = sb_pool.tile([P, D], f16, tag="osb")
                nc.scalar.activation(
                    out=osb[:], in_=out_psum[:],
                    func=mybir.ActivationFunctionType.Relu)
                nc.sync.dma_start(
                    out=OUT[d * P:d * P + rows, :], in_=osb[:rows, :])
    nc.compile()
    return nc


def _preprocess(edge_row, edge_col, edge_val):
    """Bucket edges by (core, dest block, source chunk); pad uniformly.

    Returns t_bkt and per-core (ST [P, NB*s_blk*P] f16 scatter matrices,
    COLIDX [P, NB*s_blk*8] int16 with -1 padding).
    """
    r = np.asarray(edge_row).astype(np.int64)
    c = np.asarray(edge_col).astype(np.int64)
    v = np.asarray(edge_val).astype(np.float16)

    core = r // ROWS_PER_CORE
    r_loc = r - core * ROWS_PER_CORE
    blk = r_loc // P
    rib = (r_loc - blk * P).astype(np.int64)     # row in block (0..127)
    chunk = c // CHUNK
    c_loc = (c - chunk * CHUNK).astype(np.int16)

    key = ((core * NB + blk) * NCHUNK + chunk).astype(np.int64)
    nbuckets = N_CORES * NB * NCHUNK
    counts = np.bincount(key, minlength=nbuckets)
    e_bkt = int(-(-counts.max() // P) * P)
    t_bkt = e_bkt // P
    s_blk = NCHUNK * t_bkt

    # sort by (bucket, source col): ascending gather addresses within each
    # bucket give the SDMA engines better HBM row locality
    order = np.lexsort((c, key))
    key_sorted = key[order]
    starts = np.zeros(nbuckets, np.int64)
    np.cumsum(counts[:-1], out=starts[1:])
    rank = np.arange(N_EDGES, dtype=np.int64) - starts[key_sorted]
    pos = key_sorted * e_bkt + rank   # position in global padded edge array

    trim = os.environ.get("GCN_TRIM", "1") == "1"
    tot = nbuckets * e_bkt
    col_pad = np.full(tot, -1 if trim else 0, np.int16)
    col_pad[pos] = c_loc[order]
    if trim:
        cnts = counts.astype(np.int32)
    else:
        cnts = np.full(nbuckets, e_bkt, np.int32)
    cnts = cnts.reshape(N_CORES, NB * NCHUNK)

    # scatter matrices: for padded edge at global slot `pos`, tile slot
    # s = pos//128 within its bucket chain, partition p = pos%128,
    # column j = row-in-block. One f16 write per real edge.
    # Global tile index g_tile = pos // 128 maps to (core, blk, chunk, tile)
    # in bucket order; within a core the tiles of block d are the s_blk
    # consecutive (chunk, tile) slots -> matches the ST slab layout.
    st = np.zeros((N_CORES, NB * s_blk, P, P), np.float16)  # [core, tile, p, j]
    g_tile = pos // P
    p_part = pos % P
    core_of = g_tile // (NB * s_blk)
    tile_in_core = g_tile % (NB * s_blk)
    st[core_of, tile_in_core, p_part, rib[order]] = v[order]

    colidxs = []
    col_pad = col_pad.reshape(N_CORES, NB, s_blk * P)
    for cc in range(N_CORES):
        # COLIDX: idx i of a bucket -> partition i%16 (replicated x8), slot i//16
        ci = col_pad[cc].reshape(NB * s_blk * 8, 16).T      # [16, NB*s_blk*8]
        ci = np.broadcast_to(ci[None, :, :], (8, 16, NB * s_blk * 8))
        colidxs.append(np.ascontiguousarray(ci.reshape(P, NB * s_blk * 8)))

    # ST device layout: [p, tile, j] -> [P, NB*s_blk*P]
    sts = [np.ascontiguousarray(
        st[cc].transpose(1, 0, 2).reshape(P, NB * s_blk * P)) for cc in range(N_CORES)]
    return t_bkt, sts, colidxs, cnts


def kernel(X, edge_row, edge_col, edge_val, W, b):
    X16 = np.ascontiguousarray(np.asarray(X, dtype=np.float32).astype(np.float16))
    W = np.asarray(W, dtype=np.float32)
    b = np.asarray(b, dtype=np.float32)

    t_bkt, sts, colidxs, cnts = _preprocess(edge_row, edge_col, edge_val)
    if t_bkt not in _cache:
        _cache[t_bkt] = _build_program(t_bkt)
    nc = _cache[t_bkt]

    w_rs = np.ascontiguousarray(
        W.reshape(2, P, D).transpose(1, 0, 2)).astype(np.float16)
    bias = b.reshape(1, D).astype(np.float16)
    ones = np.ones((1, P), np.float16)
    ident = np.eye(P, dtype=np.float16)

    in_maps = []
    for cc in range(N_CORES):
        in_maps.append({
            "X": X16, "Wt": w_rs, "Bb": bias, "ONES": ones,
            "IDENT": ident,
            "ST": sts[cc], "COLIDX": colidxs[cc],
            "CNT": np.ascontiguousarray(cnts[cc].reshape(1, -1)),
        })
    kw = {}
    if TRACE_TMPDIR:
        kw = dict(trace=True, tmpdir=TRACE_TMPDIR)
    res = run_bass_kernel_spmd(nc, in_maps, core_ids=list(range(N_CORES)), **kw)
    global LAST_RESULT
    LAST_RESULT = res
    return np.concatenate(
        [res.results[cc]["OUT"] for cc in range(N_CORES)], axis=0).astype(np.float32)


# revision 5
# speedup vs baseline: 1.0443x; 1.0335x over previous
"""GCN layer on 8 Trainium2 NeuronCores — v6 (batched count reg_loads).

out = relu(A @ (X @ W) + b) computed as relu((A @ X) @ W + b) (linearity),
where A is the sparse COO matrix (edge_row <- edge_col, edge_val).

Sharding: dest rows (output) split contiguously across 8 cores (12500 each).
Edges bucketed by (core, dest-block of 128 rows, source-chunk of 20000 rows);
each bucket padded to a uniform size so one SPMD program serves all cores.

v3 vs baseline:
  - X cast to fp16; gathers move 512B/edge instead of 1KB
  - dma_gather round-robined over 4 SWDGE queues (descriptor generation
    runs on a distinct GpSimd Q7 core pair per queue) -> ~3.4x DGE
  - scatter matrices S^T (one-hot(row) * val per 128-edge tile) are
    precomputed host-side and DMA'd per dest block, replacing the slow
    per-tile DVE tensor_scalar build (~900ns each, was 90% DVE busy)
  - bucket padding indices set to -1: SWDGE trims trailing negatives,
    skipping both descriptor-gen and transfer for pad slots
  - all matmuls fp16 (1 cyc/row vs 4 for fp32), accumulate fp32 in PSUM
  - per-gather valid-index counts fed via Pool registers (CNT tensor);
    with TRIM=1, pad indices are -1 and SWDGE skips them entirely
  - OUT stored fp16, upcast to fp32 on host
"""
import os
import sys
import numpy as np

sys.path.insert(0, '/opt/trn_rl_repo')

import concourse.bass as bass          # noqa: E402
import concourse.bacc as bacc          # noqa: E402
import concourse.mybir as mybir        # noqa: E402
import concourse.tile as tile          # noqa: E402
from concourse.bass_utils import run_bass_kernel_spmd  # noqa: E402

N_NODES = 100000
N_EDGES = 3200000
D = 256
N_CORES = 8
P = 128
ROWS_PER_CORE = N_NODES // N_CORES          # 12500
NB = (ROWS_PER_CORE + P - 1) // P           # 98 dest blocks (last has 84 rows)
LAST_ROWS = ROWS_PER_CORE - (NB - 1) * P    # 84
NCHUNK = 5
CHUNK = N_NODES // NCHUNK                   # 20000 (< int16 max)
GMAX_TILES = 8                              # dma_gather limit: 1024 idxs
NQ = 4                                      # SWDGE queues
f32 = mybir.dt.float32
f16 = mybir.dt.float16
i16 = mybir.dt.int16

_cache = {}

# test-harness hooks (harness only calls kernel(); these stay inert there)
TRACE_TMPDIR = None
LAST_RESULT = None


def _build_program(t_bkt: int):
    """One SPMD program; t_bkt = tiles (of 128 edges) per (block, chunk) bucket."""
    # CNT is indexed per (block, chunk) bucket, which assumes each bucket is
    # covered by a single dma_gather call.
    assert t_bkt <= GMAX_TILES, f"per-call CNT indexing requires t_bkt<={GMAX_TILES}"
    s_blk = NCHUNK * t_bkt          # tiles per dest block

    nc = bacc.Bacc("TRN2", target_bir_lowering=False, num_swdge_queues=NQ)
    X = nc.dram_tensor("X", [N_NODES, D], f16, kind="ExternalInput")
    Wt = nc.dram_tensor("Wt", [P, 2, D], f16, kind="ExternalInput")
    Bb = nc.dram_tensor("Bb", [1, D], f16, kind="ExternalInput")
    ONES = nc.dram_tensor("ONES", [1, P], f16, kind="ExternalInput")
    IDENT = nc.dram_tensor("IDENT", [P, P], f16, kind="ExternalInput")
    ST = nc.dram_tensor("ST", [P, NB * s_blk * P], f16, kind="ExternalInput")
    COLIDX = nc.dram_tensor("COLIDX", [P, NB * s_blk * 8], i16, kind="ExternalInput")
    CNT = nc.dram_tensor("CNT", [1, NB * NCHUNK], mybir.dt.int32, kind="ExternalInput")
    OUT = nc.dram_tensor("OUT", [ROWS_PER_CORE, D], f16, kind="ExternalOutput")

    qctr = [0]

    with tile.TileContext(nc) as tc:
        with (
            tc.tile_pool(name="const", bufs=1) as const_pool,
            tc.tile_pool(name="st", bufs=3) as st_pool,
            tc.tile_pool(name="idx", bufs=3) as idx_pool,
            tc.tile_pool(name="g", bufs=3) as g_pool,
            tc.tile_pool(name="agg", bufs=2, space="PSUM") as agg_pool,
            tc.tile_pool(name="tp", bufs=2, space="PSUM") as tp_pool,
            tc.tile_pool(name="op", bufs=2, space="PSUM") as op_pool,
            tc.tile_pool(name="sb", bufs=3) as sb_pool,
        ):
            w_t = const_pool.tile([P, 2, D], f16, tag="w")
            nc.sync.dma_start(out=w_t[:], in_=Wt[:, :, :])
            bias_t = const_pool.tile([1, D], f16, tag="bias")
            nc.sync.dma_start(out=bias_t[:], in_=Bb[:, :])
            ones_t = const_pool.tile([1, P], f16, tag="ones")
            nc.sync.dma_start(out=ones_t[:], in_=ONES[:, :])
            ident_t = const_pool.tile([P, P], f16, tag="ident")
            nc.sync.dma_start(out=ident_t[:], in_=IDENT[:, :])
            cnt_t = const_pool.tile([1, NB * NCHUNK], mybir.dt.int32, tag="cnt")
            nc.sync.dma_start(out=cnt_t[:], in_=CNT[:, :])
            cregs = [nc.alloc_register(mybir.EngineType.Pool, f"cnt{c}")
                     for c in range(NCHUNK)]

            # zero-fill the gather buffers once: trailing-trimmed pad slots
            # are never written, and first-touch SBUF bytes could decode as
            # NaN in fp16 (NaN * 0 = NaN in the agg matmul).
            for _ in range(3):
                g_warm = g_pool.tile([P, s_blk, D], f16, tag="g")
                nc.vector.memset(g_warm[:], 0.0)

            for d in range(NB):
                st_t = st_pool.tile([P, s_blk, P], f16, tag="st")
                nc.sync.dma_start(
                    out=st_t[:],
                    in_=ST[:, d * s_blk * P:(d + 1) * s_blk * P])
                idx_t = idx_pool.tile([P, s_blk * 8], i16, tag="idx")
                nc.sync.dma_start(
                    out=idx_t[:], in_=COLIDX[:, d * s_blk * 8:(d + 1) * s_blk * 8])

                g_t = g_pool.tile([P, s_blk, D], f16, tag="g")
                # one batched load of all NCHUNK per-bucket counts: each
                # reg_load on the Pool sequencer serializes against all four
                # SWDGE queues, so per-call loads cost ~300ns x 490 calls
                nc.gpsimd.reg_load(
                    cregs, cnt_t[0:1, d * NCHUNK:(d + 1) * NCHUNK])
                for c in range(NCHUNK):
                    for t0 in range(0, t_bkt, GMAX_TILES):
                        tn = min(GMAX_TILES, t_bkt - t0)
                        q = qctr[0] % NQ
                        nc.gpsimd.dma_gather(
                            out_ap=g_t[:, c * t_bkt + t0:c * t_bkt + t0 + tn, :],
                            in_ap=X[c * CHUNK:(c + 1) * CHUNK, :],
                            idxs_ap=idx_t[:, (c * t_bkt + t0) * 8:(c * t_bkt + t0 + tn) * 8],
                            num_idxs=tn * P,
                            num_idxs_reg=cregs[c],
                            elem_size=D,
                            queue_num=q,
                        )
                        qctr[0] += 1

                agg_psum = agg_pool.tile([P, D], f32, tag="agg", space="PSUM")
                for s in range(s_blk):
                    nc.tensor.matmul(
                        out=agg_psum[:],
                        lhsT=st_t[:, s, :],
                        rhs=g_t[:, s, :],
                        start=(s == 0),
                        stop=(s == s_blk - 1),
                    )

                # epilogue: agg -> SBUF (f16), transpose, @W + bias, relu, out
                agg_sb = sb_pool.tile([P, D], f16, tag="aggsb")
                nc.vector.tensor_copy(out=agg_sb[:], in_=agg_psum[:])
                aggT_sb = sb_pool.tile([P, 2, P], f16, tag="aggT")
                for k in range(2):
                    tp = tp_pool.tile([P, P], f16, tag="tp", space="PSUM")
                    nc.tensor.transpose(
                        out=tp[:], in_=agg_sb[:, k * P:(k + 1) * P],
                        identity=ident_t[:])
                    nc.vector.tensor_copy(out=aggT_sb[:, k, :], in_=tp[:])

                out_psum = op_pool.tile([P, D], f32, tag="outp", space="PSUM")
                for k in range(2):
                    nc.tensor.matmul(
                        out=out_psum[:], lhsT=aggT_sb[:, k, :], rhs=w_t[:, k, :],
                        start=(k == 0), stop=False)
                nc.tensor.matmul(
                    out=out_psum[:], lhsT=ones_t[:], rhs=bias_t[:],
                    start=False, stop=True)

                rows = P if d < NB - 1 else LAST_ROWS
                osb = sb_pool.tile([P, D], f16, tag="osb")
                nc.scalar.activation(
                    out=osb[:], in_=out_psum[:],
                    func=mybir.ActivationFunctionType.Relu)
                nc.sync.dma_start(
                    out=OUT[d * P:d * P + rows, :], in_=osb[:rows, :])
    nc.compile()
    return nc


def _preprocess(edge_row, edge_col, edge_val):
    """Bucket edges by (core, dest block, source chunk); pad uniformly.

    Returns t_bkt and per-core (ST [P, NB*s_blk*P] f16 scatter matrices,
    COLIDX [P, NB*s_blk*8] int16 with -1 padding).
    """
    r = np.asarray(edge_row).astype(np.int64)
    c = np.asarray(edge_col).astype(np.int64)
    v = np.asarray(edge_val).astype(np.float16)

    core = r // ROWS_PER_CORE
    r_loc = r - core * ROWS_PER_CORE
    blk = r_loc // P
    rib = (r_loc - blk * P).astype(np.int64)     # row in block (0..127)
    chunk = c // CHUNK
    c_loc = (c - chunk * CHUNK).astype(np.int16)

    key = ((core * NB + blk) * NCHUNK + chunk).astype(np.int64)
    nbuckets = N_CORES * NB * NCHUNK
    counts = np.bincount(key, minlength=nbuckets)
    e_bkt = int(-(-counts.max() // P) * P)
    t_bkt = e_bkt // P
    s_blk = NCHUNK * t_bkt

    # sort by (bucket, source col): ascending gather addresses within each
    # bucket give the SDMA engines better HBM row locality
    order = np.lexsort((c, key))
    key_sorted = key[order]
    starts = np.zeros(nbuckets, np.int64)
    np.cumsum(counts[:-1], out=starts[1:])
    rank = np.arange(N_EDGES, dtype=np.int64) - starts[key_sorted]
    pos = key_sorted * e_bkt + rank   # position in global padded edge array

    trim = os.environ.get("GCN_TRIM", "1") == "1"
    tot = nbuckets * e_bkt
    col_pad = np.full(tot, -1 if trim else 0, np.int16)
    col_pad[pos] = c_loc[order]
    if trim:
        cnts = counts.astype(np.int32)
    else:
        cnts = np.full(nbuckets, e_bkt, np.int32)
    cnts = cnts.reshape(N_CORES, NB * NCHUNK)

    # scatter matrices: for padded edge at global slot `pos`, tile slot
    # s = pos//128 within its bucket chain, partition p = pos%128,
    # column j = row-in-block. One f16 write per real edge.
    # Global tile index g_tile = pos // 128 maps to (core, blk, chunk, tile)
    # in bucket order; within a core the tiles of block d are the s_blk
    # consecutive (chunk, tile) slots -> matches the ST slab layout.
    st = np.zeros((N_CORES, NB * s_blk, P, P), np.float16)  # [core, tile, p, j]
    g_tile = pos // P
    p_part = pos % P
    core_of = g_tile // (NB * s_blk)
    tile_in_core = g_tile % (NB * s_blk)
    st[core_of, tile_in_core, p_part, rib[order]] = v[order]

    colidxs = []
    col_pad = col_pad.reshape(N_CORES, NB, s_blk * P)
    for cc in range(N_CORES):
        # COLIDX: idx i of a bucket -> partition i%16 (replicated x8), slot i//16
        ci = col_pad[cc].reshape(NB * s_blk * 8, 16).T      # [16, NB*s_blk*8]
        ci = np.broadcast_to(ci[None, :, :], (8, 16, NB * s_blk * 8))
        colidxs.append(np.ascontiguousarray(ci.reshape(P, NB * s_blk * 8)))

    # ST device layout: [p, tile, j] -> [P, NB*s_blk*P]
    sts = [np.ascontiguousarray(
        st[cc].transpose(1, 0, 2).reshape(P, NB * s_blk * P)) for cc in range(N_CORES)]
    return t_bkt, sts, colidxs, cnts


def kernel(X, edge_row, edge_col, edge_val, W, b):
    X16 = np.ascontiguousarray(np.asarray(X, dtype=np.float32).astype(np.float16))
    W = np.asarray(W, dtype=np.float32)
    b = np.asarray(b, dtype=np.float32)

    t_bkt, sts, colidxs, cnts = _preprocess(edge_row, edge_col, edge_val)
    if t_bkt not in _cache:
        _cache[t_bkt] = _build_program(t_bkt)
    nc = _cache[t_bkt]

    w_rs = np.ascontiguousarray(
        W.reshape(2, P, D).transpose(1, 0, 2)).astype(np.float16)
    bias = b.reshape(1, D).astype(np.float16)
    ones = np.ones((1, P), np.float16)
    ident = np.eye(P, dtype=np.float16)

    in_maps = []
    for cc in range(N_CORES):
        in_maps.append({
            "X": X16, "Wt": w_rs, "Bb": bias, "ONES": ones,
            "IDENT": ident,
            "ST": sts[cc], "COLIDX": colidxs[cc],
            "CNT": np.ascontiguousarray(cnts[cc].reshape(1, -1)),
        })
    kw = {}
    if TRACE_TMPDIR:
        kw = dict(trace=True, tmpdir=TRACE_TMPDIR)
    res = run_bass_kernel_spmd(nc, in_maps, core_ids=list(range(N_CORES)), **kw)
    global LAST_RESULT
    LAST_RESULT = res
    return np.concatenate(
        [res.results[cc]["OUT"] for cc in range(N_CORES)], axis=0).astype(np.float32)


# revision 6
# speedup vs baseline: 1.2022x; 1.1512x over previous
"""GCN layer on 8 Trainium2 NeuronCores — v7 (dedup cols, no warmup memsets).

out = relu(A @ (X @ W) + b) computed as relu((A @ X) @ W + b) (linearity),
where A is the sparse COO matrix (edge_row <- edge_col, edge_val).

Sharding: dest rows (output) split contiguously across 8 cores (12500 each).
Edges bucketed by (core, dest-block of 128 rows, source-chunk of 20000 rows);
each bucket padded to a uniform size so one SPMD program serves all cores.

v3 vs baseline:
  - X cast to fp16; gathers move 512B/edge instead of 1KB
  - dma_gather round-robined over 4 SWDGE queues (descriptor generation
    runs on a distinct GpSimd Q7 core pair per queue) -> ~3.4x DGE
  - scatter matrices S^T (one-hot(row) * val per 128-edge tile) are
    precomputed host-side and DMA'd per dest block, replacing the slow
    per-tile DVE tensor_scalar build (~900ns each, was 90% DVE busy)
  - bucket padding indices set to -1: SWDGE trims trailing negatives,
    skipping both descriptor-gen and transfer for pad slots
  - all matmuls fp16 (1 cyc/row vs 4 for fp32), accumulate fp32 in PSUM
  - per-gather valid-index counts fed via Pool registers (CNT tensor);
    with TRIM=1, pad indices are -1 and SWDGE skips them entirely
  - OUT stored fp16, upcast to fp32 on host
"""
import os
import sys
import numpy as np

sys.path.insert(0, '/opt/trn_rl_repo')

import concourse.bass as bass          # noqa: E402
import concourse.bacc as bacc          # noqa: E402
import concourse.mybir as mybir        # noqa: E402
import concourse.tile as tile          # noqa: E402
from concourse.bass_utils import run_bass_kernel_spmd  # noqa: E402

N_NODES = 100000
N_EDGES = 3200000
D = 256
N_CORES = 8
P = 128
ROWS_PER_CORE = N_NODES // N_CORES          # 12500
NB = (ROWS_PER_CORE + P - 1) // P           # 98 dest blocks (last has 84 rows)
LAST_ROWS = ROWS_PER_CORE - (NB - 1) * P    # 84
NCHUNK = 5
CHUNK = N_NODES // NCHUNK                   # 20000 (< int16 max)
GMAX_TILES = 8                              # dma_gather limit: 1024 idxs
NQ = 4                                      # SWDGE queues
f32 = mybir.dt.float32
f16 = mybir.dt.float16
i16 = mybir.dt.int16

_cache = {}

# test-harness hooks (harness only calls kernel(); these stay inert there)
TRACE_TMPDIR = None
LAST_RESULT = None


def _build_program(t_bkt: int):
    """One SPMD program; t_bkt = tiles (of 128 edges) per (block, chunk) bucket."""
    # CNT is indexed per (block, chunk) bucket, which assumes each bucket is
    # covered by a single dma_gather call.
    assert t_bkt <= GMAX_TILES, f"per-call CNT indexing requires t_bkt<={GMAX_TILES}"
    s_blk = NCHUNK * t_bkt          # tiles per dest block

    nc = bacc.Bacc("TRN2", target_bir_lowering=False, num_swdge_queues=NQ)
    X = nc.dram_tensor("X", [N_NODES, D], f16, kind="ExternalInput")
    Wt = nc.dram_tensor("Wt", [P, 2, D], f16, kind="ExternalInput")
    Bb = nc.dram_tensor("Bb", [1, D], f16, kind="ExternalInput")
    ONES = nc.dram_tensor("ONES", [1, P], f16, kind="ExternalInput")
    IDENT = nc.dram_tensor("IDENT", [P, P], f16, kind="ExternalInput")
    ST = nc.dram_tensor("ST", [P, NB * s_blk * P], f16, kind="ExternalInput")
    COLIDX = nc.dram_tensor("COLIDX", [P, NB * s_blk * 8], i16, kind="ExternalInput")
    CNT = nc.dram_tensor("CNT", [1, NB * NCHUNK], mybir.dt.int32, kind="ExternalInput")
    OUT = nc.dram_tensor("OUT", [ROWS_PER_CORE, D], f16, kind="ExternalOutput")

    qctr = [0]

    with tile.TileContext(nc) as tc:
        with (
            tc.tile_pool(name="const", bufs=1) as const_pool,
            tc.tile_pool(name="st", bufs=3) as st_pool,
            tc.tile_pool(name="idx", bufs=3) as idx_pool,
            tc.tile_pool(name="g", bufs=3) as g_pool,
            tc.tile_pool(name="agg", bufs=2, space="PSUM") as agg_pool,
            tc.tile_pool(name="tp", bufs=2, space="PSUM") as tp_pool,
            tc.tile_pool(name="op", bufs=2, space="PSUM") as op_pool,
            tc.tile_pool(name="sb", bufs=3) as sb_pool,
        ):
            w_t = const_pool.tile([P, 2, D], f16, tag="w")
            nc.sync.dma_start(out=w_t[:], in_=Wt[:, :, :])
            bias_t = const_pool.tile([1, D], f16, tag="bias")
            nc.sync.dma_start(out=bias_t[:], in_=Bb[:, :])
            ones_t = const_pool.tile([1, P], f16, tag="ones")
            nc.sync.dma_start(out=ones_t[:], in_=ONES[:, :])
            ident_t = const_pool.tile([P, P], f16, tag="ident")
            nc.sync.dma_start(out=ident_t[:], in_=IDENT[:, :])
            cnt_t = const_pool.tile([1, NB * NCHUNK], mybir.dt.int32, tag="cnt")
            nc.sync.dma_start(out=cnt_t[:], in_=CNT[:, :])
            cregs = [nc.alloc_register(mybir.EngineType.Pool, f"cnt{c}")
                     for c in range(NCHUNK)]

            # no warmup memsets: the first 3 dest blocks run untrimmed (full
            # padded gathers), fully initializing the 3 g buffers; later
            # trimmed blocks leave only stale-but-finite gather data behind.

            for d in range(NB):
                st_t = st_pool.tile([P, s_blk, P], f16, tag="st")
                nc.sync.dma_start(
                    out=st_t[:],
                    in_=ST[:, d * s_blk * P:(d + 1) * s_blk * P])
                idx_t = idx_pool.tile([P, s_blk * 8], i16, tag="idx")
                nc.sync.dma_start(
                    out=idx_t[:], in_=COLIDX[:, d * s_blk * 8:(d + 1) * s_blk * 8])

                g_t = g_pool.tile([P, s_blk, D], f16, tag="g")
                # one batched load of all NCHUNK per-bucket counts: each
                # reg_load on the Pool sequencer serializes against all four
                # SWDGE queues, so per-call loads cost ~300ns x 490 calls
                nc.gpsimd.reg_load(
                    cregs, cnt_t[0:1, d * NCHUNK:(d + 1) * NCHUNK])
                for c in range(NCHUNK):
                    for t0 in range(0, t_bkt, GMAX_TILES):
                        tn = min(GMAX_TILES, t_bkt - t0)
                        q = qctr[0] % NQ
                        nc.gpsimd.dma_gather(
                            out_ap=g_t[:, c * t_bkt + t0:c * t_bkt + t0 + tn, :],
                            in_ap=X[c * CHUNK:(c + 1) * CHUNK, :],
                            idxs_ap=idx_t[:, (c * t_bkt + t0) * 8:(c * t_bkt + t0 + tn) * 8],
                            num_idxs=tn * P,
                            num_idxs_reg=cregs[c],
                            elem_size=D,
                            queue_num=q,
                        )
                        qctr[0] += 1

                agg_psum = agg_pool.tile([P, D], f32, tag="agg", space="PSUM")
                for s in range(s_blk):
                    nc.tensor.matmul(
                        out=agg_psum[:],
                        lhsT=st_t[:, s, :],
                        rhs=g_t[:, s, :],
                        start=(s == 0),
                        stop=(s == s_blk - 1),
                    )

                # epilogue: agg -> SBUF (f16), transpose, @W + bias, relu, out
                agg_sb = sb_pool.tile([P, D], f16, tag="aggsb")
                nc.vector.tensor_copy(out=agg_sb[:], in_=agg_psum[:])
                aggT_sb = sb_pool.tile([P, 2, P], f16, tag="aggT")
                for k in range(2):
                    tp = tp_pool.tile([P, P], f16, tag="tp", space="PSUM")
                    nc.tensor.transpose(
                        out=tp[:], in_=agg_sb[:, k * P:(k + 1) * P],
                        identity=ident_t[:])
                    nc.vector.tensor_copy(out=aggT_sb[:, k, :], in_=tp[:])

                out_psum = op_pool.tile([P, D], f32, tag="outp", space="PSUM")
                for k in range(2):
                    nc.tensor.matmul(
                        out=out_psum[:], lhsT=aggT_sb[:, k, :], rhs=w_t[:, k, :],
                        start=(k == 0), stop=False)
                nc.tensor.matmul(
                    out=out_psum[:], lhsT=ones_t[:], rhs=bias_t[:],
                    start=False, stop=True)

                rows = P if d < NB - 1 else LAST_ROWS
                osb = sb_pool.tile([P, D], f16, tag="osb")
                nc.scalar.activation(
                    out=osb[:], in_=out_psum[:],
                    func=mybir.ActivationFunctionType.Relu)
                nc.sync.dma_start(
                    out=OUT[d * P:d * P + rows, :], in_=osb[:rows, :])
    nc.compile()
    return nc


def _preprocess(edge_row, edge_col, edge_val):
    """Bucket edges by (core, dest block, source chunk); dedup repeated
    source cols within each bucket (their S^T rows merge); pad uniformly.

    Returns t_bkt and per-core (ST [P, NB*s_blk*P] f16, COLIDX int16 with
    -1 trailing pad, CNT int32 valid-index counts per bucket).
    """
    r = np.asarray(edge_row).astype(np.int64)
    c = np.asarray(edge_col).astype(np.int64)
    v = np.asarray(edge_val).astype(np.float16)

    core = r // ROWS_PER_CORE
    r_loc = r - core * ROWS_PER_CORE
    blk = r_loc // P
    rib = (r_loc - blk * P).astype(np.int64)     # row in block (0..127)
    chunk = c // CHUNK

    key = ((core * NB + blk) * NCHUNK + chunk).astype(np.int64)
    nbuckets = N_CORES * NB * NCHUNK

    # sort by (bucket, source col): enables dedup and gives the SDMA
    # engines ascending HBM addresses within each bucket
    order = np.lexsort((c, key))
    key_s = key[order]
    c_s = c[order]
    new_grp = np.ones(N_EDGES, bool)
    new_grp[1:] = (key_s[1:] != key_s[:-1]) | (c_s[1:] != c_s[:-1])
    grp_id = np.cumsum(new_grp) - 1              # global dedup-group id

    ucounts = np.bincount(key_s[new_grp], minlength=nbuckets)
    e_bkt = int(-(-ucounts.max() // P) * P)
    t_bkt = e_bkt // P
    s_blk = NCHUNK * t_bkt

    ustarts = np.zeros(nbuckets, np.int64)
    np.cumsum(ucounts[:-1], out=ustarts[1:])
    slot_in_bucket = grp_id - ustarts[key_s]     # per edge, shared in group
    pos = key_s * e_bkt + slot_in_bucket         # padded slot per edge

    trim = os.environ.get("GCN_TRIM", "1") == "1"
    tot = nbuckets * e_bkt
    col_pad = np.full(tot, -1 if trim else 0, np.int16)
    c_loc_s = (c_s - (c_s // CHUNK) * CHUNK).astype(np.int16)
    col_pad[pos[new_grp]] = c_loc_s[new_grp]
    if trim:
        cnts = ucounts.astype(np.int32)
    else:
        cnts = np.full(nbuckets, e_bkt, np.int32)
    cnts = cnts.reshape(N_CORES, NB, NCHUNK)
    col_pad = col_pad.reshape(N_CORES, NB, NCHUNK * e_bkt)
    if trim:
        # first 3 blocks per core run untrimmed so their full gathers
        # initialize the 3 rotating g buffers (replaces warmup memsets)
        nwarm = min(3, NB)
        col3 = col_pad[:, :nwarm]
        col3[col3 < 0] = 0
        cnts[:, :nwarm] = e_bkt
    cnts = cnts.reshape(N_CORES, NB * NCHUNK)

    # scatter matrices: edges of one dedup group share a slot; a slot row
    # can hold several (rib, val) contributions (same rib sums exactly)
    st = np.zeros((N_CORES, NB * s_blk, P, P), np.float16)
    g_tile = pos // P
    p_part = pos % P
    core_of = g_tile // (NB * s_blk)
    tile_in_core = g_tile % (NB * s_blk)
    np.add.at(st, (core_of, tile_in_core, p_part, rib[order]), v[order])

    colidxs = []
    col_pad = col_pad.reshape(N_CORES, NB, s_blk * P)
    for cc in range(N_CORES):
        ci = col_pad[cc].reshape(NB * s_blk * 8, 16).T      # [16, NB*s_blk*8]
        ci = np.broadcast_to(ci[None, :, :], (8, 16, NB * s_blk * 8))
        colidxs.append(np.ascontiguousarray(ci.reshape(P, NB * s_blk * 8)))

    sts = [np.ascontiguousarray(
        st[cc].transpose(1, 0, 2).reshape(P, NB * s_blk * P)) for cc in range(N_CORES)]
    return t_bkt, sts, colidxs, cnts


def kernel(X, edge_row, edge_col, edge_val, W, b):
    X16 = np.ascontiguousarray(np.asarray(X, dtype=np.float32).astype(np.float16))
    W = np.asarray(W, dtype=np.float32)
    b = np.asarray(b, dtype=np.float32)

    t_bkt, sts, colidxs, cnts = _preprocess(edge_row, edge_col, edge_val)
    if t_bkt not in _cache:
        _cache[t_bkt] = _build_program(t_bkt)
    nc = _cache[t_bkt]

    w_rs = np.ascontiguousarray(
        W.reshape(2, P, D).transpose(1, 0, 2)).astype(np.float16)
    bias = b.reshape(1, D).astype(np.float16)
    ones = np.ones((1, P), np.float16)
    ident = np.eye(P, dtype=np.float16)

    in_maps = []
    for cc in range(N_CORES):
        in_maps.append({
            "X": X16, "Wt": w_rs, "Bb": bias, "ONES": ones,
            "IDENT": ident,
            "ST": sts[cc], "COLIDX": colidxs[cc],
            "CNT": np.ascontiguousarray(cnts[cc].reshape(1, -1)),
        })
    kw = {}
    if TRACE_TMPDIR:
        kw = dict(trace=True, tmpdir=TRACE_TMPDIR)
    res = run_bass_kernel_spmd(nc, in_maps, core_ids=list(range(N_CORES)), **kw)
    global LAST_RESULT
    LAST_RESULT = res
    return np.concatenate(
        [res.results[cc]["OUT"] for cc in range(N_CORES)], axis=0).astype(np.float32)


# revision 7
# speedup vs baseline: 1.2134x; 1.0093x over previous
"""GCN layer on 8 Trainium2 NeuronCores — v7 (dedup cols, no warmup memsets).

out = relu(A @ (X @ W) + b) computed as relu((A @ X) @ W + b) (linearity),
where A is the sparse COO matrix (edge_row <- edge_col, edge_val).

Sharding: dest rows (output) split contiguously across 8 cores (12500 each).
Edges bucketed by (core, dest-block of 128 rows, source-chunk of 20000 rows);
each bucket padded to a uniform size so one SPMD program serves all cores.

vs the fp32 single-queue baseline (4.15 ms -> ~1.14 ms on-device):
  - X cast to fp16; gathers move 512B/edge instead of 1KB
  - dma_gather round-robined over 4 SWDGE queues (descriptor generation
    runs on a distinct GpSimd Q7 core pair per queue) -> ~3.2x DGE
  - scatter matrices S^T (one-hot(row) * val per 128-edge tile) are
    precomputed host-side and DMA'd per dest block, replacing the slow
    per-tile DVE tensor_scalar build (~900ns each, was 90% DVE busy)
  - repeated source cols within a bucket dedup into one gather slot
    (their S^T rows merge); dropped t_bkt 8 -> 7 for this graph
  - bucket pad indices are -1 with exact valid counts fed to SWDGE via
    Pool registers (one batched reg_load per block): descriptor-gen and
    transfer skip pad slots. First 3 blocks run untrimmed so their full
    gathers initialize the 3 rotating gather buffers (no memsets).
  - all matmuls fp16 (1 cyc/row vs 4 for fp32), accumulate fp32 in PSUM
  - OUT stored fp16, upcast to fp32 on host
Remaining bottleneck: GpSimd SWDGE descriptor generation, ~98% busy
(~0.6us serial decode per call x 490 calls + ~8ns/index across 4 queues);
the 1024-index call cap and int16 chunk indices make this the floor.
"""
import os
import sys
import numpy as np

sys.path.insert(0, '/opt/trn_rl_repo')

import concourse.bass as bass          # noqa: E402
import concourse.bacc as bacc          # noqa: E402
import concourse.mybir as mybir        # noqa: E402
import concourse.tile as tile          # noqa: E402
from concourse.bass_utils import run_bass_kernel_spmd  # noqa: E402

N_NODES = 100000
N_EDGES = 3200000
D = 256
N_CORES = 8
P = 128
ROWS_PER_CORE = N_NODES // N_CORES          # 12500
NB = (ROWS_PER_CORE + P - 1) // P           # 98 dest blocks (last has 84 rows)
LAST_ROWS = ROWS_PER_CORE - (NB - 1) * P    # 84
NCHUNK = 5
CHUNK = N_NODES // NCHUNK                   # 20000 (< int16 max)
GMAX_TILES = 8                              # dma_gather limit: 1024 idxs
NQ = 4                                      # SWDGE queues
f32 = mybir.dt.float32
f16 = mybir.dt.float16
i16 = mybir.dt.int16

_cache = {}

# test-harness hooks (harness only calls kernel(); these stay inert there)
TRACE_TMPDIR = None
LAST_RESULT = None


def _build_program(t_bkt: int):
    """One SPMD program; t_bkt = tiles (of 128 edges) per (block, chunk) bucket."""
    # CNT is indexed per (block, chunk) bucket, which assumes each bucket is
    # covered by a single dma_gather call.
    assert t_bkt <= GMAX_TILES, f"per-call CNT indexing requires t_bkt<={GMAX_TILES}"
    s_blk = NCHUNK * t_bkt          # tiles per dest block

    nc = bacc.Bacc("TRN2", target_bir_lowering=False, num_swdge_queues=NQ)
    X = nc.dram_tensor("X", [N_NODES, D], f16, kind="ExternalInput")
    Wt = nc.dram_tensor("Wt", [P, 2, D], f16, kind="ExternalInput")
    Bb = nc.dram_tensor("Bb", [1, D], f16, kind="ExternalInput")
    ONES = nc.dram_tensor("ONES", [1, P], f16, kind="ExternalInput")
    IDENT = nc.dram_tensor("IDENT", [P, P], f16, kind="ExternalInput")
    ST = nc.dram_tensor("ST", [P, NB * s_blk * P], f16, kind="ExternalInput")
    COLIDX = nc.dram_tensor("COLIDX", [P, NB * s_blk * 8], i16, kind="ExternalInput")
    CNT = nc.dram_tensor("CNT", [1, NB * NCHUNK], mybir.dt.int32, kind="ExternalInput")
    OUT = nc.dram_tensor("OUT", [ROWS_PER_CORE, D], f16, kind="ExternalOutput")

    qctr = [0]

    with tile.TileContext(nc) as tc:
        with (
            tc.tile_pool(name="const", bufs=1) as const_pool,
            tc.tile_pool(name="st", bufs=3) as st_pool,
            tc.tile_pool(name="idx", bufs=3) as idx_pool,
            tc.tile_pool(name="g", bufs=3) as g_pool,
            tc.tile_pool(name="agg", bufs=2, space="PSUM") as agg_pool,
            tc.tile_pool(name="tp", bufs=2, space="PSUM") as tp_pool,
            tc.tile_pool(name="op", bufs=2, space="PSUM") as op_pool,
            tc.tile_pool(name="sb", bufs=3) as sb_pool,
        ):
            w_t = const_pool.tile([P, 2, D], f16, tag="w")
            nc.sync.dma_start(out=w_t[:], in_=Wt[:, :, :])
            bias_t = const_pool.tile([1, D], f16, tag="bias")
            nc.sync.dma_start(out=bias_t[:], in_=Bb[:, :])
            ones_t = const_pool.tile([1, P], f16, tag="ones")
            nc.sync.dma_start(out=ones_t[:], in_=ONES[:, :])
            ident_t = const_pool.tile([P, P], f16, tag="ident")
            nc.sync.dma_start(out=ident_t[:], in_=IDENT[:, :])
            cnt_t = const_pool.tile([1, NB * NCHUNK], mybir.dt.int32, tag="cnt")
            nc.sync.dma_start(out=cnt_t[:], in_=CNT[:, :])
            cregs = [nc.alloc_register(mybir.EngineType.Pool, f"cnt{c}")
                     for c in range(NCHUNK)]

            # no warmup memsets: the first 3 dest blocks run untrimmed (full
            # padded gathers), fully initializing the 3 g buffers; later
            # trimmed blocks leave only stale-but-finite gather data behind.

            for d in range(NB):
                st_t = st_pool.tile([P, s_blk, P], f16, tag="st")
                nc.sync.dma_start(
                    out=st_t[:],
                    in_=ST[:, d * s_blk * P:(d + 1) * s_blk * P])
                idx_t = idx_pool.tile([P, s_blk * 8], i16, tag="idx")
                nc.sync.dma_start(
                    out=idx_t[:], in_=COLIDX[:, d * s_blk * 8:(d + 1) * s_blk * 8])

                g_t = g_pool.tile([P, s_blk, D], f16, tag="g")
                # one batched load of all NCHUNK per-bucket counts: each
                # reg_load on the Pool sequencer serializes against all four
                # SWDGE queues, so per-call loads cost ~300ns x 490 calls
                nc.gpsimd.reg_load(
                    cregs, cnt_t[0:1, d * NCHUNK:(d + 1) * NCHUNK])
                for c in range(NCHUNK):
                    for t0 in range(0, t_bkt, GMAX_TILES):
                        tn = min(GMAX_TILES, t_bkt - t0)
                        q = qctr[0] % NQ
                        nc.gpsimd.dma_gather(
                            out_ap=g_t[:, c * t_bkt + t0:c * t_bkt + t0 + tn, :],
                            in_ap=X[c * CHUNK:(c + 1) * CHUNK, :],
                            idxs_ap=idx_t[:, (c * t_bkt + t0) * 8:(c * t_bkt + t0 + tn) * 8],
                            num_idxs=tn * P,
                            num_idxs_reg=cregs[c],
                            elem_size=D,
                            queue_num=q,
                        )
                        qctr[0] += 1

                agg_psum = agg_pool.tile([P, D], f32, tag="agg", space="PSUM")
                for s in range(s_blk):
                    nc.tensor.matmul(
                        out=agg_psum[:],
                        lhsT=st_t[:, s, :],
                        rhs=g_t[:, s, :],
                        start=(s == 0),
                        stop=(s == s_blk - 1),
                    )

                # epilogue: agg -> SBUF (f16), transpose, @W + bias, relu, out
                agg_sb = sb_pool.tile([P, D], f16, tag="aggsb")
                nc.vector.tensor_copy(out=agg_sb[:], in_=agg_psum[:])
                aggT_sb = sb_pool.tile([P, 2, P], f16, tag="aggT")
                for k in range(2):
                    tp = tp_pool.tile([P, P], f16, tag="tp", space="PSUM")
                    nc.tensor.transpose(
                        out=tp[:], in_=agg_sb[:, k * P:(k + 1) * P],
                        identity=ident_t[:])
                    nc.vector.tensor_copy(out=aggT_sb[:, k, :], in_=tp[:])

                out_psum = op_pool.tile([P, D], f32, tag="outp", space="PSUM")
                for k in range(2):
                    nc.tensor.matmul(
                        out=out_psum[:], lhsT=aggT_sb[:, k, :], rhs=w_t[:, k, :],
                        start=(k == 0), stop=False)
                nc.tensor.matmul(
                    out=out_psum[:], lhsT=ones_t[:], rhs=bias_t[:],
                    start=False, stop=True)

                rows = P if d < NB - 1 else LAST_ROWS
                osb = sb_pool.tile([P, D], f16, tag="osb")
                nc.scalar.activation(
                    out=osb[:], in_=out_psum[:],
                    func=mybir.ActivationFunctionType.Relu)
                nc.sync.dma_start(
                    out=OUT[d * P:d * P + rows, :], in_=osb[:rows, :])
    nc.compile()
    return nc


def _preprocess(edge_row, edge_col, edge_val):
    """Bucket edges by (core, dest block, source chunk); dedup repeated
    source cols within each bucket (their S^T rows merge); pad uniformly.

    Returns t_bkt and per-core (ST [P, NB*s_blk*P] f16, COLIDX int16 with
    -1 trailing pad, CNT int32 valid-index counts per bucket).
    """
    r = np.asarray(edge_row).astype(np.int64)
    c = np.asarray(edge_col).astype(np.int64)
    v = np.asarray(edge_val).astype(np.float16)

    core = r // ROWS_PER_CORE
    r_loc = r - core * ROWS_PER_CORE
    blk = r_loc // P
    rib = (r_loc - blk * P).astype(np.int64)     # row in block (0..127)
    chunk = c // CHUNK

    key = ((core * NB + blk) * NCHUNK + chunk).astype(np.int64)
    nbuckets = N_CORES * NB * NCHUNK

    # sort by (bucket, source col): enables dedup and gives the SDMA
    # engines ascending HBM addresses within each bucket
    order = np.lexsort((c, key))
    key_s = key[order]
    c_s = c[order]
    new_grp = np.ones(N_EDGES, bool)
    new_grp[1:] = (key_s[1:] != key_s[:-1]) | (c_s[1:] != c_s[:-1])
    grp_id = np.cumsum(new_grp) - 1              # global dedup-group id

    ucounts = np.bincount(key_s[new_grp], minlength=nbuckets)
    e_bkt = int(-(-ucounts.max() // P) * P)
    t_bkt = e_bkt // P
    s_blk = NCHUNK * t_bkt

    ustarts = np.zeros(nbuckets, np.int64)
    np.cumsum(ucounts[:-1], out=ustarts[1:])
    slot_in_bucket = grp_id - ustarts[key_s]     # per edge, shared in group
    pos = key_s * e_bkt + slot_in_bucket         # padded slot per edge

    trim = os.environ.get("GCN_TRIM", "1") == "1"
    tot = nbuckets * e_bkt
    col_pad = np.full(tot, -1 if trim else 0, np.int16)
    c_loc_s = (c_s - (c_s // CHUNK) * CHUNK).astype(np.int16)
    col_pad[pos[new_grp]] = c_loc_s[new_grp]
    if trim:
        cnts = ucounts.astype(np.int32)
    else:
        cnts = np.full(nbuckets, e_bkt, np.int32)
    cnts = cnts.reshape(N_CORES, NB, NCHUNK)
    col_pad = col_pad.reshape(N_CORES, NB, NCHUNK * e_bkt)
    if trim:
        # first 3 blocks per core run untrimmed so their full gathers
        # initialize the 3 rotating g buffers (replaces warmup memsets)
        nwarm = min(3, NB)
        col3 = col_pad[:, :nwarm]
        col3[col3 < 0] = 0
        cnts[:, :nwarm] = e_bkt
    cnts = cnts.reshape(N_CORES, NB * NCHUNK)

    # scatter matrices: edges of one dedup group share a slot; a slot row
    # can hold several (rib, val) contributions (same rib sums exactly)
    st = np.zeros((N_CORES, NB * s_blk, P, P), np.float16)
    g_tile = pos // P
    p_part = pos % P
    core_of = g_tile // (NB * s_blk)
    tile_in_core = g_tile % (NB * s_blk)
    np.add.at(st, (core_of, tile_in_core, p_part, rib[order]), v[order])

    colidxs = []
    col_pad = col_pad.reshape(N_CORES, NB, s_blk * P)
    for cc in range(N_CORES):
        ci = col_pad[cc].reshape(NB * s_blk * 8, 16).T      # [16, NB*s_blk*8]
        ci = np.broadcast_to(ci[None, :, :], (8, 16, NB * s_blk * 8))
        colidxs.append(np.ascontiguousarray(ci.reshape(P, NB * s_blk * 8)))

    sts = [np.ascontiguousarray(
        st[cc].transpose(1, 0, 2).reshape(P, NB * s_blk * P)) for cc in range(N_CORES)]
    return t_bkt, sts, colidxs, cnts


def kernel(X, edge_row, edge_col, edge_val, W, b):
    X16 = np.ascontiguousarray(np.asarray(X, dtype=np.float32).astype(np.float16))
    W = np.asarray(W, dtype=np.float32)
    b = np.asarray(b, dtype=np.float32)

    t_bkt, sts, colidxs, cnts = _preprocess(edge_row, edge_col, edge_val)
    if t_bkt not in _cache:
        _cache[t_bkt] = _build_program(t_bkt)
    nc = _cache[t_bkt]

    w_rs = np.ascontiguousarray(
        W.reshape(2, P, D).transpose(1, 0, 2)).astype(np.float16)
    bias = b.reshape(1, D).astype(np.float16)
    ones = np.ones((1, P), np.float16)
    ident = np.eye(P, dtype=np.float16)

    in_maps = []
    for cc in range(N_CORES):
        in_maps.append({
            "X": X16, "Wt": w_rs, "Bb": bias, "ONES": ones,
            "IDENT": ident,
            "ST": sts[cc], "COLIDX": colidxs[cc],
            "CNT": np.ascontiguousarray(cnts[cc].reshape(1, -1)),
        })
    kw = {}
    if TRACE_TMPDIR:
        kw = dict(trace=True, tmpdir=TRACE_TMPDIR)
    res = run_bass_kernel_spmd(nc, in_maps, core_ids=list(range(N_CORES)), **kw)
    global LAST_RESULT
    LAST_RESULT = res
    return np.concatenate(
        [res.results[cc]["OUT"] for cc in range(N_CORES)], axis=0).astype(np.float32)
